# revision 27
# baseline (speedup 1.0000x reference)
"""Causal attention head (B=8, S=4096, dk=64, scale=1/dk) on 8 TRN2 NeuronCores.

Data-parallel: batch b -> core b (no collectives). Per core, flash-style
causal attention computed in [kv, q] orientation so that:
  - QK^T matmuls contract dk (on partitions) with q streaming (N=512),
    row-packed 2x via 64x128 PE tiling (T0 = SBUF partitions 0-63,
    T8 = partitions 64-127) since the contraction dim is only 64
  - softmax denominator comes free via a ones-column appended to v
  - PV matmuls contract kv (on partitions) with q streaming (N=512)
Scores never touch HBM. exp(x/64) is split across BOTH ScalarE (LUT exp,
with the 1/dk scale folded in) and VectorE (custom single-pass DVE op:
degree-2 polynomial + 3 squarings) over [128, 2, 512] PSUM pair-tiles.
Diagonal chunks run entirely on VectorE as ONE fused exp*mask op per chunk
(wide mask = 128-triangle followed by ones), off-diagonal pairs are
greedy-balanced between the engines by modeled busy-time. Superblocks are
processed largest-first; epilogues (ScalarE PSUM->SBUF copy -> PE-transpose
back to [q, d] -> DVE reciprocal + broadcast-multiply -> bf16 DMA out) are
emitted one superblock late so the PE starts the next superblock's QK
before the transposes.

Inputs are staged as per-consumption-order chunk tiles (first superblock's
q block + first kv chunk groups land first) so the first QK starts ~3us in
instead of waiting on the whole 2MB input stream.

Host-side shard packing per batch (layout only; all math is on-device):
  qtp [128, 8, 512] bf16 : q^T duplicated into both partition halves
  ktp [128, 16, 128] bf16 : k^T chunk 2m in partitions 0-63, 2m+1 in 64-127
  vp  [128, 32, 65] bf16 : v chunks (kv on partitions) + ones column
"""

import numpy as np
import ml_dtypes

B, S, DK = 8, 4096, 64
QB = 512           # q superblock width (PSUM bank = 512 fp32)
KB = 128           # kv chunk (partition dim)
NK = S // KB       # 32 kv chunks
NQ = S // QB       # 8 q superblocks

_cache = {}

# exp(x/64) ~= ((c0 + c1*x + c2*x^2)^2)^2)^2  (quadratic fit of exp(x/512)
# on |x|<=64, then 3 squarings). Max rel err ~7e-4 for |x|<=64.
EXP_C0, EXP_C1, EXP_C2 = 1.0, 0.001956942, 1.909212e-06

# modeled engine costs for the exp balancer (measured on HW)
SC_PAIR = 1113.0   # ScalarE exp over [128, 2, 512] from PSUM
DV_PAIR = 1192.0   # VectorE EXP_P8_ANT over [128, 2, 512]
SC_EPI = 682.0     # ScalarE epilogue copy
DV_EPI = 605.0     # VectorE epilogue recip + broadcast mult


def _pin_sha(op):
    import re

    for ver in ("v3",):
        try:
            op.compile(ver)
        except ValueError as e:
            m = re.search(r'uops_sha\["' + ver + r'"\]="([0-9a-f]+)"', str(e))
            if not m:
                raise
            op.uops_sha[ver] = m.group(1)
            op.compile(ver)


def _register_exp_ops():
    """Custom single-pass DVE ops:
    EXP_P8_ANT:  out = sq(sq(sq(c2*x^2 + c1*x + c0)))          ~ exp(x/64)
    EXP_P8M_ANT: out = sq(sq(sq(c2*x^2 + c1*x + c0))) * in1    fused mask
    """
    from concourse import dve_ops
    from concourse.dve_spec import Spec, Src0, Src1, C0, C1, C2, sq

    have = {o.name: o for o in dve_ops.OPS}
    if "EXP_P8_ANT" in have:
        return have["EXP_P8_ANT"], have["EXP_P8M_ANT"]
    poly = sq(sq(sq((Src0 * C2 + C1) * Src0 + C0)))

    def ref(in0, in1, s0, s1, imm2):
        return ((((in0 * imm2 + s1) * in0 + s0) ** 2) ** 2) ** 2

    spec_e = Spec(body=poly, reference=ref)
    spec_m = Spec(
        body=poly * Src1,
        reference=lambda in0, in1, s0, s1, imm2: ref(in0, in1, s0, s1, imm2)
        * in1,
    )
    ops = []
    for name, spec in (("EXP_P8_ANT", spec_e), ("EXP_P8M_ANT", spec_m)):
        op = dve_ops.DveOp(name, spec, subdim=False, uops_sha={})
        dve_ops.OPS.append(op)
        dve_ops.CUSTOM_DVE_SPECS[name] = spec
        dve_ops._SUB_OPCODE_FOR_NAME[name] = (
            max(dve_ops._SUB_OPCODE_FOR_NAME.values()) + 1
        )
        _pin_sha(op)
        ops.append(op)
    return ops[0], ops[1]


def _build():
    from concourse.bacc import Bacc
    from concourse import tile, masks, bass
    import concourse.mybir as mybir

    exp_op, expm_op = _register_exp_ops()

    f32 = mybir.dt.float32
    bf16 = mybir.dt.bfloat16

    nc = Bacc(None, target_bir_lowering=False)
    qt_d = nc.dram_tensor("qtp", [128, NQ, QB], bf16, kind="ExternalInput")
    kt_d = nc.dram_tensor("ktp", [128, NK // 2, KB], bf16, kind="ExternalInput")
    vp_d = nc.dram_tensor("vp", [KB, NK, DK + 1], bf16, kind="ExternalInput")
    out_d = nc.dram_tensor("out", [S, DK], bf16, kind="ExternalOutput")

    with tile.TileContext(nc) as tc:
        with (
            tc.tile_pool(name="const", bufs=1) as constp,
            tc.tile_pool(name="inp", bufs=1) as inp,
            tc.tile_pool(name="attn", bufs=6) as attnp,
            tc.tile_pool(name="nv", bufs=2) as nvp,
            tc.tile_pool(name="trp", bufs=2) as trp,
            tc.tile_pool(name="outp", bufs=2) as outp,
            tc.tile_pool(name="rp", bufs=4) as rpp,
            tc.tile_pool(name="qk_ps", bufs=3, space="PSUM") as qkps,
            tc.tile_pool(name="pv_ps", bufs=2, space="PSUM") as pvps,
        ):
            # wide causal mask: cols 0-127 = lower triangle (keep where
            # qf - p >= 0), cols 128-511 = all ones.
            cmw = constp.tile([128, QB], bf16)
            nc.gpsimd.memset(cmw[:], 1.0)
            nc.gpsimd.affine_select(
                out=cmw[:, 0:KB],
                in_=cmw[:, 0:KB],
                pattern=[[1, KB]],
                compare_op=mybir.AluOpType.is_ge,
                fill=0.0,
                base=0,
                channel_multiplier=-1,
            )

            # input tiles: a small leading slice (first superblock's q block,
            # first 4 kv chunks) lands via 3 quick DMAs so the first QK
            # starts early; the remainder streams behind in 3 bulk DMAs
            # (each dma_start costs ~620ns of serial descriptor-gen on the
            # sync engine, so fewer+bigger wins once the pipe is primed).
            qt7 = inp.tile([128, QB], bf16, name="qt7")
            kta = inp.tile([128, 2, KB], bf16, name="kta")
            vpa = inp.tile([KB, 4, DK + 1], bf16, name="vpa")
            ktb = inp.tile([128, 6, KB], bf16, name="ktb")
            vpb = inp.tile([KB, 12, DK + 1], bf16, name="vpb")
            ktc = inp.tile([128, 8, KB], bf16, name="ktc")
            vpc = inp.tile([KB, 16, DK + 1], bf16, name="vpc")
            qtr = inp.tile([128, NQ - 1, QB], bf16, name="qtr")
            nc.sync.dma_start(out=qt7[:], in_=qt_d[:, NQ - 1, :])
            nc.sync.dma_start(out=kta[:], in_=kt_d[:, 0:2, :])
            nc.sync.dma_start(out=vpa[:], in_=vp_d[:, 0:4, :])
            nc.sync.dma_start(out=ktb[:], in_=kt_d[:, 2:8, :])
            nc.sync.dma_start(out=vpb[:], in_=vp_d[:, 4:16, :])
            nc.sync.dma_start(out=ktc[:], in_=kt_d[:, 8:16, :])
            nc.sync.dma_start(out=vpc[:], in_=vp_d[:, 16:32, :])
            nc.sync.dma_start(out=qtr[:], in_=qt_d[:, 0:NQ - 1, :])

            def qt_ap(I, h, c0):
                t = qt7 if I == NQ - 1 else qtr[:, I, :]
                return t[h * 64:(h + 1) * 64, c0:QB]

            def kt_ap(jj, h):
                m = jj // 2
                if m < 2:
                    t = kta[:, m, :]
                elif m < 8:
                    t = ktb[:, m - 2, :]
                else:
                    t = ktc[:, m - 8, :]
                return t[h * 64:(h + 1) * 64, :]

            def vp_ap(jj):
                if jj < 4:
                    return vpa[:, jj, :]
                if jj < 16:
                    return vpb[:, jj - 4, :]
                return vpc[:, jj - 16, :]

            busy = [0.0, 0.0]   # modeled ns: [ScalarE, VectorE]

            def emit_exp_pair(qk, at, I, p):
                """exp for chunk pair (2p, 2p+1) of superblock I."""
                j0 = 2 * p
                if j0 + 1 < 4 * I:
                    # off-diagonal pair: one plain op on the lighter engine
                    if busy[0] + SC_PAIR <= busy[1] + DV_PAIR:
                        busy[0] += SC_PAIR
                        nc.scalar.activation(
                            out=at[:, :, :], in_=qk[:, :, :],
                            func=mybir.ActivationFunctionType.Exp,
                            scale=1.0 / DK,
                        )
                    else:
                        busy[1] += DV_PAIR
                        nc.vector._custom_dve(
                            exp_op, out=at[:, :, :], in0=qk[:, :, :],
                            s0=EXP_C0, s1=EXP_C1, imm2=EXP_C2,
                        )
                    return
                # diagonal pair: per-chunk exp with triangle masking. Either
                # VectorE fused exp*mask (wide mask covers triangle +
                # trailing plain in one op) or, when ScalarE is lighter,
                # ScalarE exp + a GpSimd affine_select zeroing the triangle
                # (GpSimd is otherwise idle).
                for u in range(2):
                    jj = j0 + u
                    v0 = (jj - 4 * I) * KB
                    W = QB - v0
                    d_cost = (120.0 + W) / 0.96
                    s_cost = (352.0 + W) / 1.2
                    if busy[0] + s_cost <= busy[1] + d_cost:
                        busy[0] += s_cost
                        nc.scalar.activation(
                            out=at[:, u, v0:QB], in_=qk[:, u, v0:QB],
                            func=mybir.ActivationFunctionType.Exp,
                            scale=1.0 / DK,
                        )
                        nc.gpsimd.affine_select(
                            out=at[:, u, v0:v0 + KB],
                            in_=at[:, u, v0:v0 + KB],
                            pattern=[[1, KB]],
                            compare_op=mybir.AluOpType.is_ge,
                            fill=0.0,
                            base=0,
                            channel_multiplier=-1,
                        )
                    else:
                        busy[1] += d_cost
                        nc.vector._custom_dve(
                            expm_op,
                            out=at[:, u, v0:QB], in0=qk[:, u, v0:QB],
                            in1=cmw[:, 0:W],
                            s0=EXP_C0, s1=EXP_C1, imm2=EXP_C2,
                        )

            def emit_epi_a(I, pv):
                # stage A: PSUM -> SBUF copy on ScalarE (bf16, padded to 80
                # partitions for the DMA xbar's 16-row granularity)
                nv = nvp.tile([80, QB], bf16, tag="nv")
                # memset rows 64-79 (gpsimd needs 16-aligned partition
                # start); the copy then overwrites row 64 with the real dens
                nc.gpsimd.memset(nv[DK:80, :], 0.0)
                nc.scalar.copy(nv[0:DK + 1, :], pv[:])
                busy[0] += SC_EPI
                return nv

            def emit_epi_b(I, nv):
                # stage B (a group later, so nv is surely resident): one DMA
                # xbar transpose back to [q, d]: trt[p,t,d] = nv[d, t*128+p]
                trt = trp.tile([128, 4, 80], bf16, tag="trt")
                nc.sync.dma_start_transpose(trt[:], nv[:])
                return trt

            def emit_epi_c(I, trt):
                # stage C (another group later, transpose DMA has landed):
                # normalize and DMA out
                r4 = rpp.tile([128, 4], f32, tag="r")
                nc.vector.reciprocal(r4[:], trt[:, :, DK])
                ot = outp.tile([128, 4, DK], bf16, tag="ot")
                # broadcast-multiply: ot[p,t,d] = trt[p,t,d] * r4[p,t]
                in0 = trt[:, :, 0:DK]
                in1 = r4[:].rearrange("p (f o) -> p f o", o=1)
                in0b, in1b = bass.broadcast_tensor_aps(in0, in1)
                nc.vector.tensor_tensor(
                    ot[:], in0b, in1b, mybir.AluOpType.mult
                )
                busy[1] += DV_EPI
                nc.sync.dma_start(
                    out=out_d[I * QB:(I + 1) * QB].rearrange(
                        "(t p) d -> p t d", p=128
                    ),
                    in_=ot[:],
                )

            def emit_qk(qk, I, p):
                j0 = 2 * p
                for u in range(2):
                    jj = j0 + u
                    v0 = max(0, (jj - 4 * I) * KB)
                    nc.tensor.matmul(
                        qk[:, u, v0:QB],
                        kt_ap(jj, jj % 2),
                        qt_ap(I, jj % 2, v0),
                        start=True, stop=True,
                        tile_position=(64 * (jj % 2), 0),
                    )

            def emit_pv(pv, at, I, p, C):
                j0 = 2 * p
                for u in range(2):
                    jj = j0 + u
                    v0 = max(0, (jj - 4 * I) * KB)
                    nc.tensor.matmul(
                        pv[:, v0:QB],
                        vp_ap(jj),
                        at[:, u, v0:QB],
                        start=(jj == 0), stop=(jj == C - 1),
                    )

            # Software-pipelined emission. Per 2-pair group: QK matmuls and
            # exp ops are emitted immediately; the PV matmuls are deferred
            # one group so that when the PE's FIFO queue reaches them their
            # exp inputs completed long ago (no head-of-queue stall, no
            # pipeline-restart penalty). Epilogues are likewise staged: the
            # ScalarE copy one group after the superblock's last PV, the
            # transposes/normalize/DMA another group later.
            pend_pv = None     # (pv, ats, I, ps, C) PV batch awaiting emit
            pend_a = None      # (I, pv) awaiting epilogue stage A
            pend_b = None      # (I, nv) awaiting epilogue stage B
            pend_c = None      # (I, trt) awaiting epilogue stage C
            # big superblocks first: deep pipeline from the start, shallow
            # ones drain at the end
            for I in reversed(range(NQ)):
                C = 4 * I + 4          # causal kv chunks for this superblock
                P = C // 2
                pv = pvps.tile([DK + 1, QB], f32, tag="pvtr")
                sizes = []
                left = P
                while left:
                    if left == 4:
                        sizes += [2, 2]
                        left = 0
                    elif left >= 3:
                        sizes.append(3)
                        left -= 3
                    else:
                        sizes.append(left)
                        left = 0
                base = 0
                for sz in sizes:
                    ps = tuple(range(base, base + sz))
                    base += sz
                    qks, ats = [], []
                    for p in ps:
                        qk = qkps.tile([128, 2, QB], f32, tag="qk")
                        at = attnp.tile([128, 2, QB], bf16, tag="at")
                        qks.append(qk)
                        ats.append(at)
                        emit_qk(qk, I, p)
                    for qk, at, p in zip(qks, ats, ps):
                        emit_exp_pair(qk, at, I, p)
                    if pend_c is not None:
                        emit_epi_c(*pend_c)
                        pend_c = None
                    if pend_b is not None:
                        bI, bnv = pend_b
                        pend_c = (bI, emit_epi_b(bI, bnv))
                        pend_b = None
                    if pend_pv is not None:
                        opv, oats, oI, ops_, oC = pend_pv
                        for oat, op_ in zip(oats, ops_):
                            emit_pv(opv, oat, oI, op_, oC)
                        pend_pv = None
                    if pend_a is not None:
                        aI, apv = pend_a
                        pend_b = (aI, emit_epi_a(aI, apv))
                        pend_a = None
                    pend_pv = (pv, ats, I, ps, C)
                pend_a = (I, pv)
            # drain the pipeline tail
            if pend_c is not None:
                emit_epi_c(*pend_c)
                pend_c = None
            if pend_b is not None:
                bI, bnv = pend_b
                pend_c = (bI, emit_epi_b(bI, bnv))
                pend_b = None
            if pend_pv is not None:
                opv, oats, oI, ops_, oC = pend_pv
                for oat, op_ in zip(oats, ops_):
                    emit_pv(opv, oat, oI, op_, oC)
            if pend_a is not None:
                aI, apv = pend_a
                pend_b = (aI, emit_epi_a(aI, apv))
            if pend_c is not None:
                emit_epi_c(*pend_c)
                pend_c = None
            if pend_b is not None:
                bI, bnv = pend_b
                pend_c = (bI, emit_epi_b(bI, bnv))
            if pend_c is not None:
                emit_epi_c(*pend_c)

    nc.compile()
    return nc


def _get_nc():
    if "nc" not in _cache:
        _cache["nc"] = _build()
    return _cache["nc"]


def make_in_maps(q, k, v):
    bf = ml_dtypes.bfloat16
    q = np.asarray(q)
    k = np.asarray(k)
    v = np.asarray(v)
    in_maps = []
    for b in range(B):
        qt = np.ascontiguousarray(q[b].T).astype(bf)          # [64, 4096]
        qtp = np.concatenate([qt, qt], axis=0)                # [128, 4096]
        kt = np.ascontiguousarray(k[b].T).astype(bf)          # [64, 4096]
        ktc = kt.reshape(DK, NK, KB)                          # [64, 32, 128]
        ktp = np.empty((128, NK // 2, KB), dtype=bf)
        ktp[0:DK] = ktc[:, 0::2, :]
        ktp[DK:128] = ktc[:, 1::2, :]
        vpk = np.empty((KB, NK, DK + 1), dtype=bf)
        vpk[:, :, 0:DK] = v[b].reshape(NK, KB, DK).transpose(1, 0, 2)
        vpk[:, :, DK] = 1.0
        in_maps.append({
            "qtp": np.ascontiguousarray(qtp.reshape(128, NQ, QB)),
            "ktp": np.ascontiguousarray(ktp),
            "vp": np.ascontiguousarray(vpk),
        })
    return in_maps


def kernel(q, k, v):
    from concourse.bass_utils import run_bass_kernel_spmd

    nc = _get_nc()
    in_maps = make_in_maps(q, k, v)
    res = run_bass_kernel_spmd(nc, in_maps, core_ids=list(range(B)))
    out = np.stack([np.asarray(res.results[i]["out"]) for i in range(B)], axis=0)
    return out.astype(np.float32)


# revision 30
# speedup vs baseline: 1.1917x; 1.1917x over previous
"""Causal attention head (B=8, S=4096, dk=64, scale=1/dk) on 8 TRN2 NeuronCores.

Data-parallel: batch b -> core b (no collectives). Per core, flash-style
causal attention computed in [kv, q] orientation so that:
  - QK^T matmuls contract dk (on partitions) with q streaming (N=512),
    row-packed 2x via 64x128 PE tiling (T0 = SBUF partitions 0-63,
    T8 = partitions 64-127) since the contraction dim is only 64
  - softmax denominator comes free via a ones-column appended to v
  - PV matmuls contract kv (on partitions) with q streaming (N=512)
Scores never touch HBM. exp(x/64) is split across BOTH ScalarE (LUT exp,
with the 1/dk scale folded in) and VectorE (custom single-pass DVE op:
degree-2 polynomial + 3 squarings) over [128, 2, 512] PSUM pair-tiles.
Diagonal chunks run entirely on VectorE as ONE fused exp*mask op per chunk
(wide mask = 128-triangle followed by ones), off-diagonal pairs are
greedy-balanced between the engines by modeled busy-time. Superblocks are
processed largest-first; epilogues (ScalarE PSUM->SBUF copy -> PE-transpose
back to [q, d] -> DVE reciprocal + broadcast-multiply -> bf16 DMA out) are
emitted one superblock late so the PE starts the next superblock's QK
before the transposes.

Inputs are staged as per-consumption-order chunk tiles (first superblock's
q block + first kv chunk groups land first) so the first QK starts ~3us in
instead of waiting on the whole 2MB input stream.

Host-side shard packing per batch (layout only; all math is on-device):
  qtp [128, 8, 512] bf16 : q^T duplicated into both partition halves
  ktp [128, 16, 128] bf16 : k^T chunk 2m in partitions 0-63, 2m+1 in 64-127
  vp  [128, 32, 65] bf16 : v chunks (kv on partitions) + ones column
"""

import numpy as np
import ml_dtypes

B, S, DK = 8, 4096, 64
QB = 512           # q superblock width (PSUM bank = 512 fp32)
KB = 128           # kv chunk (partition dim)
NK = S // KB       # 32 kv chunks
NQ = S // QB       # 8 q superblocks

_cache = {}

# exp(x/64) ~= ((c0 + c1*x + c2*x^2)^2)^2)^2  (quadratic fit of exp(x/512)
# on |x|<=64, then 3 squarings). Max rel err ~7e-4 for |x|<=64.
EXP_C0, EXP_C1, EXP_C2 = 1.0, 0.001956942, 1.909212e-06

# modeled engine costs for the exp balancer (measured on HW)
SC_PAIR = 1113.0   # ScalarE exp over [128, 2, 512] from PSUM
DV_PAIR = 1192.0   # VectorE EXP_P8_ANT over [128, 2, 512]
SC_EPI = 682.0     # ScalarE epilogue copy
DV_EPI = 605.0     # VectorE epilogue recip + broadcast mult


def _pin_sha(op):
    import re

    for ver in ("v3",):
        try:
            op.compile(ver)
        except ValueError as e:
            m = re.search(r'uops_sha\["' + ver + r'"\]="([0-9a-f]+)"', str(e))
            if not m:
                raise
            op.uops_sha[ver] = m.group(1)
            op.compile(ver)


def _register_exp_ops():
    """Custom single-pass DVE ops:
    EXP_P8_ANT:  out = sq(sq(sq(c2*x^2 + c1*x + c0)))          ~ exp(x/64)
    EXP_P8M_ANT: out = sq(sq(sq(c2*x^2 + c1*x + c0))) * in1    fused mask
    """
    from concourse import dve_ops
    from concourse.dve_spec import Spec, Src0, Src1, C0, C1, C2, sq

    have = {o.name: o for o in dve_ops.OPS}
    if "EXP_P8_ANT" in have:
        return have["EXP_P8_ANT"], have["EXP_P8M_ANT"]
    poly = sq(sq(sq((Src0 * C2 + C1) * Src0 + C0)))

    def ref(in0, in1, s0, s1, imm2):
        return ((((in0 * imm2 + s1) * in0 + s0) ** 2) ** 2) ** 2

    spec_e = Spec(body=poly, reference=ref)
    spec_m = Spec(
        body=poly * Src1,
        reference=lambda in0, in1, s0, s1, imm2: ref(in0, in1, s0, s1, imm2)
        * in1,
    )
    ops = []
    for name, spec in (("EXP_P8_ANT", spec_e), ("EXP_P8M_ANT", spec_m)):
        op = dve_ops.DveOp(name, spec, subdim=False, uops_sha={})
        dve_ops.OPS.append(op)
        dve_ops.CUSTOM_DVE_SPECS[name] = spec
        dve_ops._SUB_OPCODE_FOR_NAME[name] = (
            max(dve_ops._SUB_OPCODE_FOR_NAME.values()) + 1
        )
        _pin_sha(op)
        ops.append(op)
    return ops[0], ops[1]


def _build():
    from concourse.bacc import Bacc
    from concourse import tile, masks, bass
    import concourse.mybir as mybir

    exp_op, expm_op = _register_exp_ops()

    f32 = mybir.dt.float32
    bf16 = mybir.dt.bfloat16

    nc = Bacc(None, target_bir_lowering=False)
    qt_d = nc.dram_tensor("qtp", [128, NQ, QB], bf16, kind="ExternalInput")
    kt_d = nc.dram_tensor("ktp", [128, NK // 2, KB], bf16, kind="ExternalInput")
    vp_d = nc.dram_tensor("vp", [KB, NK, DK + 1], bf16, kind="ExternalInput")
    out_d = nc.dram_tensor("out", [S, DK], bf16, kind="ExternalOutput")

    with tile.TileContext(nc) as tc:
        with (
            tc.tile_pool(name="const", bufs=1) as constp,
            tc.tile_pool(name="inp", bufs=1) as inp,
            tc.tile_pool(name="attn", bufs=8) as attnp,
            tc.tile_pool(name="nv", bufs=2) as nvp,
            tc.tile_pool(name="outp", bufs=2) as outp,
            tc.tile_pool(name="rp", bufs=4) as rpp,
            tc.tile_pool(name="qk_ps", bufs=3, space="PSUM") as qkps,
            tc.tile_pool(name="pv_ps", bufs=2, space="PSUM") as pvps,
        ):
            ident = constp.tile([128, 128], f32)
            masks.make_identity(nc, ident[:])

            # wide causal mask: cols 0-127 = lower triangle (keep where
            # qf - p >= 0), cols 128-511 = all ones.
            cmw = constp.tile([128, QB], bf16)
            nc.gpsimd.memset(cmw[:], 1.0)
            nc.gpsimd.affine_select(
                out=cmw[:, 0:KB],
                in_=cmw[:, 0:KB],
                pattern=[[1, KB]],
                compare_op=mybir.AluOpType.is_ge,
                fill=0.0,
                base=0,
                channel_multiplier=-1,
            )

            # input tiles: a small leading slice (first superblock's q block,
            # first 4 kv chunks) lands via 3 quick DMAs so the first QK
            # starts early; the remainder streams behind in 3 bulk DMAs
            # (each dma_start costs ~620ns of serial descriptor-gen on the
            # sync engine, so fewer+bigger wins once the pipe is primed).
            qt7 = inp.tile([128, QB], bf16, name="qt7")
            kta = inp.tile([128, 2, KB], bf16, name="kta")
            vpa = inp.tile([KB, 4, DK + 1], bf16, name="vpa")
            ktb = inp.tile([128, 6, KB], bf16, name="ktb")
            vpb = inp.tile([KB, 12, DK + 1], bf16, name="vpb")
            ktc = inp.tile([128, 8, KB], bf16, name="ktc")
            vpc = inp.tile([KB, 16, DK + 1], bf16, name="vpc")
            qtr = inp.tile([128, NQ - 1, QB], bf16, name="qtr")
            nc.sync.dma_start(out=qt7[:], in_=qt_d[:, NQ - 1, :])
            nc.sync.dma_start(out=kta[:], in_=kt_d[:, 0:2, :])
            nc.sync.dma_start(out=vpa[:], in_=vp_d[:, 0:4, :])
            nc.sync.dma_start(out=ktb[:], in_=kt_d[:, 2:8, :])
            nc.sync.dma_start(out=vpb[:], in_=vp_d[:, 4:16, :])
            nc.sync.dma_start(out=ktc[:], in_=kt_d[:, 8:16, :])
            nc.sync.dma_start(out=vpc[:], in_=vp_d[:, 16:32, :])
            nc.sync.dma_start(out=qtr[:], in_=qt_d[:, 0:NQ - 1, :])

            # PE warmup: dummy fp32 matmuls on the identity while the input
            # DMA streams. The HAM clock gate needs ~3.4us of PE activity to
            # unthrottle from 1.2 to 2.4 GHz; without this the first ~8 real
            # matmuls run at half speed. Uses the qk PSUM ring's first slot
            # (cycles back into the pool untouched).
            warm = qkps.tile([128, 2, QB], f32, tag="qk", name="warm")
            for w in range(10):
                nc.tensor.matmul(
                    warm[:, 0, 0:128], ident[:], ident[:],
                    start=True, stop=True,
                )

            def qt_ap(I, h, c0):
                t = qt7 if I == NQ - 1 else qtr[:, I, :]
                return t[h * 64:(h + 1) * 64, c0:QB]

            def kt_ap(jj, h):
                m = jj // 2
                if m < 2:
                    t = kta[:, m, :]
                elif m < 8:
                    t = ktb[:, m - 2, :]
                else:
                    t = ktc[:, m - 8, :]
                return t[h * 64:(h + 1) * 64, :]

            def vp_ap(jj):
                if jj < 4:
                    return vpa[:, jj, :]
                if jj < 16:
                    return vpb[:, jj - 4, :]
                return vpc[:, jj - 16, :]

            busy = [0.0, 0.0]   # modeled ns: [ScalarE, VectorE]

            def emit_exp_pair(qk, at, I, p):
                """exp for chunk pair (2p, 2p+1) of superblock I."""
                j0 = 2 * p
                if j0 + 1 < 4 * I:
                    # off-diagonal pair: one plain op on the lighter engine
                    if busy[0] + SC_PAIR <= busy[1] + DV_PAIR:
                        busy[0] += SC_PAIR
                        nc.scalar.activation(
                            out=at[:, :, :], in_=qk[:, :, :],
                            func=mybir.ActivationFunctionType.Exp,
                            scale=1.0 / DK,
                        )
                    else:
                        busy[1] += DV_PAIR
                        nc.vector._custom_dve(
                            exp_op, out=at[:, :, :], in0=qk[:, :, :],
                            s0=EXP_C0, s1=EXP_C1, imm2=EXP_C2,
                        )
                    return
                # diagonal pair: per-chunk exp with triangle masking. Either
                # VectorE fused exp*mask (wide mask covers triangle +
                # trailing plain in one op) or, when ScalarE is lighter,
                # ScalarE exp + a GpSimd affine_select zeroing the triangle
                # (GpSimd is otherwise idle).
                for u in range(2):
                    jj = j0 + u
                    v0 = (jj - 4 * I) * KB
                    W = QB - v0
                    d_cost = (120.0 + W) / 0.96
                    s_cost = (352.0 + W) / 1.2
                    if busy[0] + s_cost <= busy[1] + d_cost:
                        busy[0] += s_cost
                        nc.scalar.activation(
                            out=at[:, u, v0:QB], in_=qk[:, u, v0:QB],
                            func=mybir.ActivationFunctionType.Exp,
                            scale=1.0 / DK,
                        )
                        nc.gpsimd.affine_select(
                            out=at[:, u, v0:v0 + KB],
                            in_=at[:, u, v0:v0 + KB],
                            pattern=[[1, KB]],
                            compare_op=mybir.AluOpType.is_ge,
                            fill=0.0,
                            base=0,
                            channel_multiplier=-1,
                        )
                    else:
                        busy[1] += d_cost
                        nc.vector._custom_dve(
                            expm_op,
                            out=at[:, u, v0:QB], in0=qk[:, u, v0:QB],
                            in1=cmw[:, 0:W],
                            s0=EXP_C0, s1=EXP_C1, imm2=EXP_C2,
                        )

            def emit_epi_a(I, pv):
                # stage A: PSUM -> SBUF copy on ScalarE
                nv = nvp.tile([DK + 1, QB], f32, tag="nv")
                nc.scalar.copy(nv[:], pv[:])
                busy[0] += SC_EPI
                return nv

            def emit_epi_b(I, nv):
                # stage B (a group later, so nv is surely resident):
                # transpose back to [q, d], normalize, DMA out
                tr4 = pvps.tile([128, 4, DK + 1], f32, tag="pvtr")
                for t in range(4):
                    nc.tensor.matmul(
                        tr4[:, t, :],
                        nv[:, t * 128:(t + 1) * 128],
                        ident[0:DK + 1, 0:DK + 1],
                        is_transpose=True,
                    )
                r4 = rpp.tile([128, 4], f32, tag="r")
                nc.vector.reciprocal(r4[:], tr4[:, :, DK])
                ot = outp.tile([128, 4, DK], bf16, tag="ot")
                # broadcast-multiply: ot[p,t,d] = tr4[p,t,d] * r4[p,t]
                in0 = tr4[:, :, 0:DK]
                in1 = r4[:].rearrange("p (f o) -> p f o", o=1)
                in0b, in1b = bass.broadcast_tensor_aps(in0, in1)
                nc.vector.tensor_tensor(
                    ot[:], in0b, in1b, mybir.AluOpType.mult
                )
                busy[1] += DV_EPI
                nc.sync.dma_start(
                    out=out_d[I * QB:(I + 1) * QB].rearrange(
                        "(t p) d -> p t d", p=128
                    ),
                    in_=ot[:],
                )

            def emit_qk(qk, I, p):
                j0 = 2 * p
                for u in range(2):
                    jj = j0 + u
                    v0 = max(0, (jj - 4 * I) * KB)
                    nc.tensor.matmul(
                        qk[:, u, v0:QB],
                        kt_ap(jj, jj % 2),
                        qt_ap(I, jj % 2, v0),
                        start=True, stop=True,
                        tile_position=(64 * (jj % 2), 0),
                    )

            def emit_pv(pv, at, I, p, C):
                j0 = 2 * p
                for u in range(2):
                    jj = j0 + u
                    v0 = max(0, (jj - 4 * I) * KB)
                    nc.tensor.matmul(
                        pv[:, v0:QB],
                        vp_ap(jj),
                        at[:, u, v0:QB],
                        start=(jj == 0), stop=(jj == C - 1),
                    )

            # Software-pipelined emission. Per 2-pair group: QK matmuls and
            # exp ops are emitted immediately; the PV matmuls are deferred
            # one group so that when the PE's FIFO queue reaches them their
            # exp inputs completed long ago (no head-of-queue stall, no
            # pipeline-restart penalty). Epilogues are likewise staged: the
            # ScalarE copy one group after the superblock's last PV, the
            # transposes/normalize/DMA another group later.
            pend_pv = None     # (pv, ats, I, ps, C) PV batch awaiting emit
            pend_a = None      # (I, pv) awaiting epilogue stage A
            pend_b = None      # (I, nv) awaiting epilogue stage B
            # big superblocks first: deep pipeline from the start, shallow
            # ones drain at the end
            for I in reversed(range(NQ)):
                C = 4 * I + 4          # causal kv chunks for this superblock
                P = C // 2
                pv = pvps.tile([DK + 1, QB], f32, tag="pvtr")
                sizes = []
                left = P
                while left:
                    if left == 4:
                        sizes += [2, 2]
                        left = 0
                    elif left >= 3:
                        sizes.append(3)
                        left -= 3
                    else:
                        sizes.append(left)
                        left = 0
                base = 0
                for sz in sizes:
                    ps = tuple(range(base, base + sz))
                    base += sz
                    qks, ats = [], []
                    for p in ps:
                        qk = qkps.tile([128, 2, QB], f32, tag="qk")
                        at = attnp.tile([128, 2, QB], bf16, tag="at")
                        qks.append(qk)
                        ats.append(at)
                        emit_qk(qk, I, p)
                    for qk, at, p in zip(qks, ats, ps):
                        emit_exp_pair(qk, at, I, p)
                    if pend_b is not None:
                        emit_epi_b(*pend_b)
                        pend_b = None
                    if pend_pv is not None:
                        opv, oats, oI, ops_, oC = pend_pv
                        for oat, op_ in zip(oats, ops_):
                            emit_pv(opv, oat, oI, op_, oC)
                        pend_pv = None
                    if pend_a is not None:
                        aI, apv = pend_a
                        pend_b = (aI, emit_epi_a(aI, apv))
                        pend_a = None
                    pend_pv = (pv, ats, I, ps, C)
                pend_a = (I, pv)
            # drain the pipeline tail
            if pend_b is not None:
                emit_epi_b(*pend_b)
                pend_b = None
            if pend_pv is not None:
                opv, oats, oI, ops_, oC = pend_pv
                for oat, op_ in zip(oats, ops_):
                    emit_pv(opv, oat, oI, op_, oC)
            if pend_a is not None:
                aI, apv = pend_a
                pend_b = (aI, emit_epi_a(aI, apv))
            if pend_b is not None:
                emit_epi_b(*pend_b)

    nc.compile()
    return nc


def _get_nc():
    if "nc" not in _cache:
        _cache["nc"] = _build()
    return _cache["nc"]


def make_in_maps(q, k, v):
    bf = ml_dtypes.bfloat16
    q = np.asarray(q)
    k = np.asarray(k)
    v = np.asarray(v)
    in_maps = []
    for b in range(B):
        qt = np.ascontiguousarray(q[b].T).astype(bf)          # [64, 4096]
        qtp = np.concatenate([qt, qt], axis=0)                # [128, 4096]
        kt = np.ascontiguousarray(k[b].T).astype(bf)          # [64, 4096]
        ktc = kt.reshape(DK, NK, KB)                          # [64, 32, 128]
        ktp = np.empty((128, NK // 2, KB), dtype=bf)
        ktp[0:DK] = ktc[:, 0::2, :]
        ktp[DK:128] = ktc[:, 1::2, :]
        vpk = np.empty((KB, NK, DK + 1), dtype=bf)
        vpk[:, :, 0:DK] = v[b].reshape(NK, KB, DK).transpose(1, 0, 2)
        vpk[:, :, DK] = 1.0
        in_maps.append({
            "qtp": np.ascontiguousarray(qtp.reshape(128, NQ, QB)),
            "ktp": np.ascontiguousarray(ktp),
            "vp": np.ascontiguousarray(vpk),
        })
    return in_maps


def kernel(q, k, v):
    from concourse.bass_utils import run_bass_kernel_spmd

    nc = _get_nc()
    in_maps = make_in_maps(q, k, v)
    res = run_bass_kernel_spmd(nc, in_maps, core_ids=list(range(B)))
    out = np.stack([np.asarray(res.results[i]["out"]) for i in range(B)], axis=0)
    return out.astype(np.float32)
